# revision 15
# baseline (speedup 1.0000x reference)
"""Trainium2 Bass kernel for the ternary-MLP decoder.

  h   = tanh(x @ (s1 * tern(w1 - scale*n1)) + b1)
  out = (h @ (s2 * tern(w2 - scale*n2)) + b2).reshape(-1, 3, 32, 32)

Strategy (8 NeuronCores, Megatron tensor-parallel over D_H):
  - core c owns h-columns [c*2048, (c+1)*2048): w1/s1/b1 column shard,
    w2 row shard. Full batch on every core.
  - All matmuls computed in transposed space: hT = t1c.T @ xT,
    poutT = t2c.T @ hT. Host passes xT (fp16) and reassembles outT.
  - Ternarization on-device in exact fp32 compares (bit-identical to
    the reference); ternary weights stored fp8e4 (exact for {-1,0,1}).
    Ternarize runs column-block-major so the first output-column block
    of t2 is ready ~25us after its DMA, letting L2 start long before
    the full 50MB w2/n2 stream lands.
  - L2 hybrid precision: 8 of 16 contraction k-tiles use fp16 moving
    h (exact-ish), the last 8 use fp8e4 h via DoubleRow pairs (2
    k-tiles per PE pass -> ~2x those matmuls). Measured end-to-end
    rel err 1.81e-2 vs the fp32 reference (gate 2e-2), bf16
    stage/ReduceScatter rounding included.
  - s2 scaling (+ b2 on the owning core only, via host-zeroed bias
    planes) is folded into the PSUM->SBUF eviction on the Vector
    engine (bf16 out), so after the ReduceScatter the result goes
    DRAM->DRAM straight into outT (bf16; host upcasts).
  - Cross-core reduction: per 512-wide batch chunk, three bf16
    ReduceScatters of 1024 rows each; core c owns channels
    [g*1024 + c*128, +128) which the host inverts when assembling.
"""

import os
from contextlib import ExitStack

import numpy as np

import concourse.tile as tile
from concourse import bacc, mybir
from concourse.bass_utils import run_bass_kernel_spmd

F32 = mybir.dt.float32
FP16 = mybir.dt.float16
BF16 = mybir.dt.bfloat16
FP8 = mybir.dt.float8e4

# Problem dims (hardcoded per contract).
B, DIN, DH, DOUT = 4096, 1024, 16384, 3 * 32 * 32
W = 8   # cores
N8 = 8  # of the 16 L2 k-tiles per core, how many carry fp8 h (DoubleRow)

# Results of the last traced run (for test harness inspection).
LAST_RUN = None


def build_decoder_nc(
    scale: float,
    b: int = B,
    din: int = DIN,
    dh: int = DH,
    dout: int = DOUT,
    w: int = W,
    cb: int = 512,
    n8: int = N8,
):
    """Build the per-core Bass program (same program for all cores; the
    per-core shards arrive as inputs)."""
    P = 128
    hsh = dh // w        # h columns owned by this core
    osh = dout // w      # outT rows owned after ReduceScatter
    nkt1 = din // P      # L1 contraction tiles
    nkt2 = hsh // P      # L2 contraction tiles (== L1 output m-tiles)
    nm1 = hsh // P       # L1 output tiles (hT rows / P)
    nm2 = dout // P      # L2 output tiles (outT rows / P)
    nor = osh // P       # post-RS row tiles
    nch = b // cb        # batch chunks
    n16 = nkt2 - n8      # fp16 L2 k-tiles
    assert n8 % 2 == 0 and 0 <= n8 <= nkt2
    assert din % P == 0 and hsh % P == 0 and dout % P == 0 and osh % P == 0
    assert b % cb == 0 and cb <= 512

    nc = bacc.Bacc(None, num_devices=w)

    xT = nc.dram_tensor("xT", [din, b], FP16, kind="ExternalInput")
    w1c = nc.dram_tensor("w1c", [din, hsh], F32, kind="ExternalInput")
    n1c = nc.dram_tensor("n1c", [din, hsh], F32, kind="ExternalInput")
    w2c = nc.dram_tensor("w2c", [hsh, dout], F32, kind="ExternalInput")
    n2c = nc.dram_tensor("n2c", [hsh, dout], F32, kind="ExternalInput")
    s1c = nc.dram_tensor("s1c", [P, nm1], F32, kind="ExternalInput")
    b1c = nc.dram_tensor("b1c", [P, nm1], F32, kind="ExternalInput")
    # Per-mo-tile scale/bias for the pre-RS fold: s2 for all 24 mo
    # tiles; b2 host-zeroed except on the owning core's tiles.
    s2c = nc.dram_tensor("s2c", [P, nm2], F32, kind="ExternalInput")
    b2c = nc.dram_tensor("b2c", [P, nm2], F32, kind="ExternalInput")
    outT = nc.dram_tensor("outT", [osh, b], BF16, kind="ExternalOutput")

    # Per-chunk DRAM buffers for the cross-core reduction.
    nrs = nor
    rs_rows = dout // nrs
    assert rs_rows % (w * P) == 0 and rs_rows // w == P
    spans = [(ch * cb, cb) for ch in range(nch)]
    partials = [
        [nc.dram_tensor(f"partial_{i}_{g}", [rs_rows, bw], BF16) for g in range(nrs)]
        for i, (_, bw) in enumerate(spans)
    ]
    rs_outs = [
        [nc.dram_tensor(f"rs_out_{i}_{g}", [P, bw], BF16) for g in range(nrs)]
        for i, (_, bw) in enumerate(spans)
    ]
    groups = [list(range(w))]
    mo_per_g = nm2 // nrs

    xT3 = xT.rearrange("(ko p) b -> p ko b", p=P)
    w1r = w1c.rearrange("(ko p) f -> p ko f", p=P)
    n1r = n1c.rearrange("(ko p) f -> p ko f", p=P)
    w2r = w2c.rearrange("(ko p) f -> p ko f", p=P)
    n2r = n2c.rearrange("(ko p) f -> p ko f", p=P)

    with tile.TileContext(nc) as tc, ExitStack() as ctx:
        consts = ctx.enter_context(tc.tile_pool(name="consts", bufs=1))
        t1p = ctx.enter_context(tc.tile_pool(name="t1", bufs=1))
        t2p = ctx.enter_context(tc.tile_pool(name="t2", bufs=1))
        wnp = ctx.enter_context(tc.tile_pool(name="wn", bufs=4))
        ltp = ctx.enter_context(tc.tile_pool(name="lt", bufs=3))
        xp = ctx.enter_context(tc.tile_pool(name="xb", bufs=4))
        h16p = ctx.enter_context(tc.tile_pool(name="h16", bufs=6))
        h8p = ctx.enter_context(tc.tile_pool(name="h8", bufs=6))
        stp = ctx.enter_context(tc.tile_pool(name="stage", bufs=8))
        ps1 = ctx.enter_context(tc.tile_pool(name="ps1", bufs=2, space="PSUM"))
        ps2 = ctx.enter_context(tc.tile_pool(name="ps2", bufs=6, space="PSUM"))

        # Per-partition scale/bias vectors (host pre-arranged as [128, m]).
        s1_sb = consts.tile([P, nm1], F32, tag="s1")
        b1_sb = consts.tile([P, nm1], F32, tag="b1")
        s2_sb = consts.tile([P, nm2], F32, tag="s2")
        b2_sb = consts.tile([P, nm2], F32, tag="b2")
        nc.sync.dma_start(s1_sb[:], s1c[:])
        nc.sync.dma_start(b1_sb[:], b1c[:])
        nc.sync.dma_start(s2_sb[:], s2c[:])
        nc.sync.dma_start(b2_sb[:], b2c[:])

        # Tiny warm-up ReduceScatter: absorb the ~100us one-off ncfw init
        # during the startup phase where the PE is idle anyway.
        warm_in = nc.dram_tensor("cc_warm_in", [w * 8, 64], F32)
        warm_out = nc.dram_tensor("cc_warm_out", [8, 64], F32)
        wz = consts.tile([8, 64], F32, tag="wz")
        nc.gpsimd.memset(wz[:], 0.0)
        for r in range(w):
            nc.sync.dma_start(warm_in[r * 8 : (r + 1) * 8, :], wz[:])
        nc.gpsimd.collective_compute(
            "ReduceScatter",
            mybir.AluOpType.add,
            replica_groups=groups,
            ins=[warm_in[:]],
            outs=[warm_out[:]],
        )

        def tern_block(dst3, w_r, n_r, nkt, cols):
            """dst3[:, kt, :] = (q>1) - (q<-1) for all kt, q = w - scale*n.

            Column-block-major: one 512-wide j-block across every k-tile
            lands in its own SBUF tile, so downstream consumers of early
            j-blocks unblock ASAP. Exact fp32 compares, identical to the
            reference semantics."""
            fw = cols.stop - cols.start
            for kt in range(nkt):
                wt = wnp.tile([P, fw], F32, tag="w")
                nc.sync.dma_start(wt[:], w_r[:, kt, cols])
                if scale != 0.0:
                    nt = wnp.tile([P, fw], F32, tag="n")
                    nc.sync.dma_start(nt[:], n_r[:, kt, cols])
                    if scale != 1.0:
                        nc.vector.tensor_scalar(
                            nt[:], nt[:], float(scale), None, mybir.AluOpType.mult
                        )
                    nc.vector.tensor_tensor(
                        wt[:], wt[:], nt[:], mybir.AluOpType.subtract
                    )
                lt = ltp.tile([P, fw], BF16, tag="lt")
                nc.vector.tensor_scalar(
                    lt[:], wt[:], -1.0, None, mybir.AluOpType.is_lt
                )
                # t = (q > 1) - (q < -1), fused compare+subtract.
                nc.vector.scalar_tensor_tensor(
                    dst3[:, kt, :],
                    wt[:],
                    1.0,
                    lt[:],
                    mybir.AluOpType.is_gt,
                    mybir.AluOpType.subtract,
                )

        # Resident ternary weights (fp8), one tile per 512-col j-block so
        # dependency tracking frees consumers per block:
        #   t1: 4 x [128, 8, 512], t2: 6 x [128, 16, 512].
        TF = 512
        nj1 = hsh // TF
        nj2 = dout // TF
        mo_per_j = TF // P  # output m-tiles per j-block
        t1_sb = [
            t1p.tile([P, nkt1, TF], FP8, tag=f"t1_{j}", name=f"t1_{j}")
            for j in range(nj1)
        ]
        t2_sb = [
            t2p.tile([P, nkt2, TF], FP8, tag=f"t2_{j}", name=f"t2_{j}")
            for j in range(nj2)
        ]

        # x DMA ordering: ch0-1 ahead of the w1/n1 stream (first L1 matmul
        # gates on x ch0 + t1 j-block 0), ch2-5 after it, ch6-7 after the
        # big w2/n2 stream. The xb pool has 4 buffers, so later loads also
        # wait for an earlier chunk's buffer to free (tile inserts the WAR
        # dep); all land long before their chunk needs them.
        xb_tiles = {}

        def fetch_x(ch):
            b0, bw = spans[ch]
            xb = xp.tile([P, nkt1, cb], FP16, tag="xb", name=f"xb_{ch}")[:, :, :bw]
            nc.sync.dma_start(xb[:], xT3[:, :, b0 : b0 + bw])
            xb_tiles[ch] = xb

        for ch in range(min(2, nch)):
            fetch_x(ch)

        for j in range(nj1):
            tern_block(t1_sb[j], w1r, n1r, nkt1, slice(j * TF, (j + 1) * TF))

        for ch in range(2, min(6, nch)):
            fetch_x(ch)

        for j in range(nj2):
            tern_block(t2_sb[j], w2r, n2r, nkt2, slice(j * TF, (j + 1) * TF))

        for ch in range(6, nch):
            fetch_x(ch)

        h16_t, h8_t = {}, {}

        def get_h(ch):
            if ch not in h16_t:
                bw = spans[ch][1]
                h16_t[ch] = h16p.tile(
                    [P, n16, cb], FP16, tag="h16", name=f"h16_{ch}"
                )[:, :, :bw]
                h8_t[ch] = h8p.tile([P, n8, cb], FP8, tag="h8", name=f"h8_{ch}")[
                    :, :, :bw
                ]
            return h16_t[ch], h8_t[ch]

        def l1_mtiles(ch, ms):
            """hT[m] = tanh((t1.T @ xT)[m] * s1[m] + b1[m]) for m in ms.
            m-tiles 0..n16-1 -> fp16 planes; n16.. -> fp8 planes (DR)."""
            bw = spans[ch][1]
            xb = xb_tiles[ch]
            h16, h8 = get_h(ch)
            for m in ms:
                acc = ps1.tile([P, cb], F32, tag="ps1", name=f"ps1_{ch}_{m}")[:, :bw]
                mj, mr = divmod(m, mo_per_j)
                for kt in range(nkt1):
                    nc.tensor.matmul(
                        acc[:],
                        t1_sb[mj][:, kt, mr * P : (mr + 1) * P],
                        xb[:, kt, :],
                        start=(kt == 0),
                        stop=(kt == nkt1 - 1),
                    )
                hdst = h16[:, m, :] if m < n16 else h8[:, m - n16, :]
                nc.scalar.activation(
                    hdst,
                    acc[:],
                    mybir.ActivationFunctionType.Tanh,
                    bias=b1_sb[:, m : m + 1],
                    scale=s1_sb[:, m : m + 1],
                )

        # L1 of chunks 0-1 interleaved j-block-major: each arriving t1
        # j-block unlocks 4 m-tiles x 2 chunks of PE work, so the PE is
        # not starved while the t1 stream lands.
        n_early = min(2, nch)
        for mj in range(nj1):
            for ch in range(n_early):
                l1_mtiles(ch, range(mj * mo_per_j, (mj + 1) * mo_per_j))

        for ch, (b0, bw) in enumerate(spans):
            bcols = slice(b0, b0 + bw)

            if ch >= n_early:
                l1_mtiles(ch, range(nm1))
            h16, h8 = get_h(ch)

            # L2: poutT[mo] = s2[mo] * (t2.T @ hT)[mo] (+ b2 if owner).
            for mo in range(nm2):
                acc = ps2.tile([P, cb], F32, tag="ps2", name=f"ps2_{ch}_{mo}")[:, :bw]
                oj, orr = divmod(mo, mo_per_j)
                osl = slice(orr * P, (orr + 1) * P)
                for kt in range(n16):
                    nc.tensor.matmul(
                        acc[:],
                        t2_sb[oj][:, kt, osl],
                        h16[:, kt, :],
                        start=(kt == 0),
                        stop=False,
                    )
                for p in range(n8 // 2):
                    nc.tensor.matmul(
                        acc[:],
                        t2_sb[oj][:, n16 + 2 * p : n16 + 2 * p + 2, osl],
                        h8[:, 2 * p : 2 * p + 2, :],
                        start=False,
                        stop=(p == n8 // 2 - 1),
                        perf_mode=mybir.MatmulPerfMode.DoubleRow,
                    )
                st = stp.tile([P, cb], BF16, tag="st", name=f"st_{ch}_{mo}")[:, :bw]
                nc.vector.tensor_scalar(
                    st[:],
                    acc[:],
                    s2_sb[:, mo : mo + 1],
                    b2_sb[:, mo : mo + 1],
                    mybir.AluOpType.mult,
                    mybir.AluOpType.add,
                )
                g, mg = divmod(mo, mo_per_g)
                nc.sync.dma_start(
                    partials[ch][g][mg * P : (mg + 1) * P, :bw], st[:]
                )
                if mg == mo_per_g - 1:
                    nc.gpsimd.collective_compute(
                        "ReduceScatter",
                        mybir.AluOpType.add,
                        replica_groups=groups,
                        ins=[partials[ch][g][:]],
                        outs=[rs_outs[ch][g][:]],
                    )
                    # Owned slab is final (s2/b2 pre-folded): DRAM->DRAM.
                    nc.sync.dma_start(
                        outT[g * P : (g + 1) * P, bcols], rs_outs[ch][g][:]
                    )

    nc.compile()
    return nc


def _chan_perm(c, w=W, dout=DOUT):
    """Output channels owned by core c, in shard-row order: for each RS
    group g (w*128 rows), core c gets rows [c*128, (c+1)*128)."""
    P = 128
    rs_rows = w * P
    nrs = dout // rs_rows
    return np.concatenate(
        [np.arange(g * rs_rows + c * P, g * rs_rows + (c + 1) * P) for g in range(nrs)]
    )


def _shard_inputs(x, w1, s1, b1, w2, s2, b2, n1, n2, w=W, dh=DH, dout=DOUT):
    P = 128
    hsh = dh // w
    nm1 = hsh // P
    nm2 = dout // P
    xT = np.ascontiguousarray(x.T).astype(np.float16)
    s2all = np.ascontiguousarray(s2.reshape(nm2, P).T)
    in_maps = []
    for c in range(w):
        hs = slice(c * hsh, (c + 1) * hsh)
        # b2 applied pre-RS exactly once: zero except this core's mo tiles.
        b2all = np.zeros((nm2, P), np.float32)
        for mo in range(nm2):
            if mo % w == c:
                b2all[mo] = b2[mo * P : (mo + 1) * P]
        in_maps.append(
            {
                "xT": xT,
                "w1c": np.ascontiguousarray(w1[:, hs]),
                "n1c": np.ascontiguousarray(n1[:, hs]),
                "w2c": np.ascontiguousarray(w2[hs, :]),
                "n2c": np.ascontiguousarray(n2[hs, :]),
                "s1c": np.ascontiguousarray(s1[hs].reshape(nm1, P).T),
                "b1c": np.ascontiguousarray(b1[hs].reshape(nm1, P).T),
                "s2c": s2all,
                "b2c": np.ascontiguousarray(b2all.T),
            }
        )
    return in_maps


_NC_CACHE = {}


def kernel(**inputs) -> np.ndarray:
    global LAST_RUN
    x = np.asarray(inputs["x"], dtype=np.float32)
    w1 = np.asarray(inputs["w1"], dtype=np.float32)
    s1 = np.asarray(inputs["s1"], dtype=np.float32)
    b1 = np.asarray(inputs["b1"], dtype=np.float32)
    w2 = np.asarray(inputs["w2"], dtype=np.float32)
    s2 = np.asarray(inputs["s2"], dtype=np.float32)
    b2 = np.asarray(inputs["b2"], dtype=np.float32)
    n1 = np.asarray(inputs["n1"], dtype=np.float32)
    n2 = np.asarray(inputs["n2"], dtype=np.float32)
    scale = float(np.asarray(inputs["scale"]))

    key = scale
    if key not in _NC_CACHE:
        _NC_CACHE[key] = build_decoder_nc(scale)
    nc = _NC_CACHE[key]

    in_maps = _shard_inputs(x, w1, s1, b1, w2, s2, b2, n1, n2)
    trace = bool(int(os.environ.get("KERNEL_TRACE", "0")))
    res = run_bass_kernel_spmd(
        nc, in_maps, core_ids=list(range(W)), trace=trace
    )
    LAST_RUN = res

    outT = np.empty((DOUT, B), np.float32)
    for c in range(W):
        outT[_chan_perm(c)] = np.asarray(res.results[c]["outT"]).astype(np.float32)
    out = np.ascontiguousarray(outT.T).reshape(B, 3, 32, 32).astype(np.float32)
    return out
